# revision 37
# baseline (speedup 1.0000x reference)
"""Fused MHA block (qkvg proj + biased masked softmax + sigmoid gating +
out proj + residual + LayerNorm) for one TRN2 chip — fp8 DoubleRow, v2.

Sharding: data parallel over batch. B=8 -> 8 NeuronCores, one batch element
per core, no collectives. Weights replicated.

Changes vs v1 (112.3us):
  * Bias+mask injection is FUSED into the scores matmul's second DoubleRow
    slot instead of a separate identity matmul: lhsT = [k-block | C*I] via a
    per-kt strided AP over a KI tile whose row 8 holds C*I; rhs = [q | BT_kt]
    via a per-kt strided AP over a QB tile whose rows 1..8 hold the
    TRANSPOSED bias (BT[p,kt,q] = (gamma*bias[q,kt*128+p]+OFF)*SU, masked ->
    -240).  Halves the scores PE cost (one DR matmul per (kt, 512q) chunk).
  * Scores PSUM is one [128,2,N] tile (4 banks); exp runs once per kt-PAIR
    over [128,2048], amortizing the ACT access-latency overhead (32 exps of
    2048 instead of 64 of 1024).
  * Gate path: t=exp(-g) (ACT) then ONE custom-DVE AFFINE_MUL_REDUCE
    den2 = (16t+16)*denom and r2 = 1/den2 (DVE), ATT = av*r2 (Pool).
    Removes the Pool affine + separate sigmoid reciprocal + one multiply.
  * rstd = exp(-0.5*ln(var+eps')): Ln+Exp live in the same ACT table
    (natural_log_exp_and_others) as the softmax Exp -> ONE table load total.
  * Residual rides the ff matmul as fp8 hi/lo DoubleRow (x16 = hi+lo, both
    slots through a stride-0 identity lhsT) instead of bf16 identity:
    halves residual PE cost, same precision (~0.4%).
  * LN normalize split ACT/Pool per 512-chunk to balance engines.
  * v-copies moved DVE -> Pool.
  * No zero-slot q/k tiles -> no big Pool memsets.

Scale ledger (fp8 ranges; e4m3 max finite = 240):
    Wq,Wk *8 ; Wv,Wg *64 ; x *1        -> q_ps=8q k_ps=8k v_ps=64v g_ps=64g
    exp arg = sc_ps * ES, ES=1/(64*sqrt(128)); bias via BT=(gamma*b+OFF)*SU,
        SU = 1/(ES*C), C=128, OFF=-3 (exp <= e^~2.5, fits fp8)
    sig path: t=exp(-g_ps/64)=e^-g (ACT); den2=(16t+16)*denom (DVE AMR);
        r2=1/den2 (DVE)
    ATT = av_ps * r2 = 4*attv*sig   (av_ps = 64*denom*attv)
    W_ff *4 -> ff_ps = 16*ff ; x16 = hi+lo fp8 ; h_ps = 16*(x+ff)
    eps' = 256*eps ; rstd = exp(-0.5*ln(var+eps'))
"""

import math

import numpy as np
import ml_dtypes

import concourse.bass as bass
import concourse.mybir as mybir
import concourse.tile as tile
from concourse import bacc
from concourse.bass_utils import run_bass_kernel_spmd

B, N, D, H, DH = 8, 1024, 1024, 8, 128
KT = D // 128
KTP = KT // 2
LN_EPS = 1e-5

F32 = mybir.dt.float32
BF16 = mybir.dt.bfloat16
FP8 = mybir.dt.float8e4
DR = mybir.MatmulPerfMode.DoubleRow
FP8NP = ml_dtypes.float8_e4m3
AF = mybir.ActivationFunctionType

SQ = 8.0        # q,k weight prescale
SV = 64.0       # v,g weight prescale
CID = 128.0     # identity-slot constant
ES = 1.0 / (SQ * SQ * math.sqrt(DH))     # exp() scale on scores psum
SU = 1.0 / (ES * CID)                    # bias prescale into BT
OFF = -3.0      # score offset (softmax-invariant), keeps exp in fp8 range
SA = 16.0       # h_ps = SA*(x+ff)
EPS2 = LN_EPS * SA * SA
NPAR = 4
# Schraudolph int-exp constants (DVE offload of one exp chunk per head):
# exp(x) ~ bitcast_f32(int32(x*2^23/ln2 + 127*2^23 - C)); C tuned for
# mean-relative error ~1.8%, below fp8's own 6% quantization step.
XA = float(2**23 / math.log(2.0))
XB = float(127 * 2**23 - 366393)

_cache = {}


def _ident(nc, ap2d, fill):
    """diag(fill) into a zeroed [128,128] view."""
    nc.gpsimd.memset(ap2d, 0.0)
    nc.gpsimd.affine_select(
        out=ap2d,
        in_=ap2d,
        compare_op=mybir.AluOpType.not_equal,
        fill=fill,
        base=0,
        pattern=[[-1, 128]],
        channel_multiplier=1,
    )


def _build(flags):
    general_gamma, use_bff, use_lng, use_lnb = flags
    nc = bacc.Bacc("TRN2", target_bir_lowering=False)

    xt8_d = nc.dram_tensor("xt8", [128, KT, N], FP8, kind="ExternalInput")
    bt_shape = [H, 128, KT, N] if general_gamma else [128, KT, N]
    bt_d = nc.dram_tensor("bt", bt_shape, FP8, kind="ExternalInput")
    watt_d = nc.dram_tensor("watt", [H, 128, 4, KTP, 2, 128], FP8, kind="ExternalInput")
    wff_d = nc.dram_tensor("wff", [128, H, D], FP8, kind="ExternalInput")
    xhl_d = nc.dram_tensor("xhl", [KT, 128, 2, D], FP8, kind="ExternalInput")
    if use_bff:
        bff_d = nc.dram_tensor("bff", [1, D], F32, kind="ExternalInput")
    if use_lng:
        lng_d = nc.dram_tensor("lng", [1, D], F32, kind="ExternalInput")
    if use_lnb:
        lnb_d = nc.dram_tensor("lnb", [1, D], F32, kind="ExternalInput")
    out_d = nc.dram_tensor("out", [N, D], BF16, kind="ExternalOutput")

    with tile.TileContext(nc) as tc:
        with (
            tc.tile_pool(name="singles", bufs=1) as singles,
            tc.tile_pool(name="sb_w", bufs=3) as sb_w,
            tc.tile_pool(name="sb_sig", bufs=4) as sb_sig,
            tc.tile_pool(name="sb_v", bufs=3) as sb_v,
            tc.tile_pool(name="sb_dr", bufs=4) as sb_dr,
            tc.tile_pool(name="sb_st", bufs=4) as sb_st,
            tc.tile_pool(name="sb_o", bufs=3) as sb_o,
            tc.tile_pool(name="ps_sc", bufs=2, space="PSUM") as ps_sc,
            tc.tile_pool(name="ps_pr", bufs=2, space="PSUM") as ps_pr,
            tc.tile_pool(name="ps_tl", bufs=2, space="PSUM") as ps_tl,
        ):
            # ---- constants / resident tensors ----
            XT8 = singles.tile([128, KT, N], FP8, tag="XT8")
            wt0 = singles.tile([128, 4, KTP, 2, 128], FP8, tag="wt0")
            # head-0 critical path first: x^T half 0, head-0 q/k weights,
            # first bias rows, x^T half 1, then the rest.
            nc.sync.dma_start(out=XT8[:, :, 0:512], in_=xt8_d[:, :, 0:512])
            nc.sync.dma_start(out=wt0[:, 0:2, :, :, :], in_=watt_d[0, :, 0:2])

            NQB = 2
            QB = [singles.tile([128, 1 + KT, N], FP8, tag=f"QB{p}", name=f"QB{p}")
                  for p in range(NQB)]
            KI = [singles.tile([128, KT + 1, 128], FP8, tag=f"KI{p}", name=f"KI{p}")
                  for p in range(NPAR)]
            PT2 = [singles.tile([128, KT, N], FP8, tag=f"PT{p}", name=f"PT{p}")
                   for p in range(NPAR)]
            if not general_gamma:
                nc.sync.dma_start(out=QB[0][:, 1:3, :], in_=bt_d[:, 0:2, :])
            nc.sync.dma_start(out=XT8[:, :, 512:N], in_=xt8_d[:, :, 512:N])
            nc.sync.dma_start(out=wt0[:, 3, :, :, :], in_=watt_d[0, :, 3])
            nc.sync.dma_start(out=wt0[:, 2, :, :, :], in_=watt_d[0, :, 2])
            if not general_gamma:
                nc.sync.dma_start(out=QB[0][:, 3 : 1 + KT, :], in_=bt_d[:, 2:KT, :])
                nc.sync.dma_start(out=QB[1][:, 1 : 1 + KT, :], in_=bt_d[:, :, :])

            WFF8 = singles.tile([128, H, D], FP8, tag="WFF8")
            XR8 = singles.tile([128, KT, 2, D], FP8, tag="XR8")
            ATT = singles.tile([128, H, N], FP8, tag="ATT")
            ones2 = singles.tile([128, 2, 128], FP8, tag="ones2")
            nc.gpsimd.memset(ones2, 1.0)
            eps_t = singles.tile([128, 1], F32, tag="eps_t")
            nc.gpsimd.memset(eps_t, EPS2)
            ID1 = singles.tile([128, 128], FP8, tag="ID1")
            _ident(nc, ID1, 1.0)
            for p in range(NPAR):
                _ident(nc, KI[p][:, KT, :], CID)
            if use_bff:
                bffb = singles.tile([128, D], F32, tag="bffb")
                nc.sync.dma_start(
                    out=bffb,
                    in_=bass.AP(tensor=bff_d, offset=0, ap=[[0, 128], [1, D]]),
                )
            if use_lng:
                lngb = singles.tile([128, D], F32, tag="lngb")
                nc.sync.dma_start(
                    out=lngb,
                    in_=bass.AP(tensor=lng_d, offset=0, ap=[[0, 128], [1, D]]),
                )
            if use_lnb:
                lnbb = singles.tile([128, D], F32, tag="lnbb")
                nc.sync.dma_start(
                    out=lnbb,
                    in_=bass.AP(tensor=lnb_d, offset=0, ap=[[0, 128], [1, D]]),
                )

            # strided-AP helpers (slot-1 rides a different row of the tile)
            def ki_lhs(par, kt):
                t = KI[par][:, 0, :]
                return bass.AP(
                    tensor=t.tensor,
                    offset=kt * 128,
                    ap=[[(KT + 1) * 128, 128], [(KT - kt) * 128, 2], [1, 128]],
                )

            def qb_rhs(par, kt, c):
                t = QB[par][:, 0, :]
                return bass.AP(
                    tensor=t.tensor,
                    offset=c * 512,
                    ap=[[(1 + KT) * N, 128], [(1 + kt) * N, 2], [1, 512]],
                )

            def id_res():
                return bass.AP(
                    tensor=ID1[:, :].tensor,
                    offset=0,
                    ap=[[128, 128], [0, 2], [1, 128]],
                )

            # ---- per-head attention ----
            wts = {0: wt0}

            def prefetch_wt(hh):
                if hh < H and hh not in wts:
                    w = sb_w.tile([128, 4, KTP, 2, 128], FP8, tag="wt", name="wt")
                    nc.sync.dma_start(out=w, in_=watt_d[hh])
                    wts[hh] = w

            prefetch_wt(1)

            # PE p-state warmup: keep the tensor engine continuously busy on
            # throwaway all-ones matmuls while the first DMAs land, so real
            # matmuls start at full clock (3us ramp).
            warm = ps_tl.tile([128, 128], F32, tag="ps_tl", name="warm")
            for _ in range(56):
                nc.tensor.matmul(
                    warm, ones2, ones2[:, :, 0:128],
                    start=True, stop=True, perf_mode=DR,
                )

            def proj(wt, j, c, out_ap, start=True, stop=True):
                for ktp in range(KTP):
                    nc.tensor.matmul(
                        out_ap,
                        wt[:, j, ktp, :, :],
                        XT8[:, 2 * ktp : 2 * ktp + 2, c * 512 : (c + 1) * 512],
                        start=start and (ktp == 0),
                        stop=stop and (ktp == KTP - 1),
                        perf_mode=DR,
                    )

            def qk_phase(h):
                """q -> QB row 0, k -> KI rows 0..7, chunk-by-chunk."""
                parq, par = h % NQB, h % NPAR
                wt = wts[h]
                for c in range(2):
                    for j in (0, 1):
                        pr = ps_pr.tile([128, 512], F32, tag="ps_pr", name=f"pr{j}{c}")
                        proj(wt, j, c, pr)
                        if j == 0:
                            o_ap = QB[parq][:, 0, c * 512 : (c + 1) * 512]
                        else:
                            o_ap = KI[par][:, 4 * c : 4 * c + 4, :]
                        nc.vector.tensor_copy(out=o_ap, in_=pr)

            def score_kt(h, kt):
                par, parq = h % NPAR, h % NQB
                sc = ps_sc.tile([128, N], F32, tag="ps_sc", name=f"s{kt}")
                for c in range(2):
                    nc.tensor.matmul(
                        sc[:, c * 512 : (c + 1) * 512],
                        ki_lhs(par, kt),
                        qb_rhs(parq, kt, c),
                        start=True,
                        stop=True,
                        perf_mode=DR,
                    )
                    if h == 0 and kt < 2:
                        # head 0 cold-start: per-chunk exps so ACT starts
                        # as soon as the first 512 q columns land
                        nc.scalar.activation(
                            out=PT2[par][:, kt, c * 512 : (c + 1) * 512],
                            in_=sc[:, c * 512 : (c + 1) * 512],
                            func=AF.Exp,
                            scale=ES,
                        )
                if not (h == 0 and kt < 2):
                    nc.scalar.activation(
                        out=PT2[par][:, kt, :], in_=sc, func=AF.Exp, scale=ES
                    )

            sigs = {}

            def head_prologue(h):
                """q/k projections, kt0/kt1 scores+exp, and the gate chain
                for head h — emitted before the PREVIOUS head's dn/av so
                both PE and ACT roll straight through the head boundary."""
                qk_phase(h)
                score_kt(h, 0)
                score_kt(h, 1)
                sig_t = sb_sig.tile([128, N], BF16, tag="sig_t", name="sig_t")
                sig_r = sb_sig.tile([128, N], BF16, tag="sig_r", name="sig_r")
                gp = ps_sc.tile([128, N], F32, tag="ps_sc", name="gp")
                wt = wts[h]
                for c in range(2):
                    proj(wt, 3, c, gp[:, c * 512 : (c + 1) * 512])
                nc.scalar.activation(
                    out=sig_t, in_=gp, func=AF.Exp, scale=-1.0 / SV
                )
                nc.gpsimd.tensor_scalar(
                    sig_r, sig_t, SA, SA,
                    mybir.AluOpType.mult, mybir.AluOpType.add,
                )
                with nc.allow_low_precision(reason="gate bf16 ~0.4%"):
                    nc.vector.reciprocal(sig_r, sig_r)
                sigs[h] = sig_r

            prefetch_wt(1)
            head_prologue(0)
            for h in range(H):
                par = h % NPAR
                parq = h % NQB
                QBp, KIp, PT = QB[parq], KI[par], PT2[par]
                wt = wts.pop(h)
                prefetch_wt(h + 2)
                if h == 6:
                    nc.sync.dma_start(out=WFF8, in_=wff_d[:, :, :])
                if general_gamma:
                    nc.sync.dma_start(out=QBp[:, 1 : 1 + KT, :], in_=bt_d[h])

                if h == H - 1:
                    # pre-start nt0's ff: residual + head-pairs 0..2 now
                    # (ps_pr is idle here); pair 3 lands in the FF loop.
                    ff_pre = []
                    for c in range(2):
                        ff = ps_pr.tile(
                            [128, 512], F32, tag="ps_pr", name=f"ffp{c}"
                        )
                        ff_pre.append(ff)
                        nc.tensor.matmul(
                            ff,
                            id_res(),
                            XR8[:, 0, :, c * 512 : (c + 1) * 512],
                            start=True,
                            stop=False,
                            perf_mode=DR,
                        )
                        for fp4 in range(KTP - 1):
                            nc.tensor.matmul(
                                ff,
                                ATT[:, 2 * fp4 : 2 * fp4 + 2, 0:128],
                                WFF8[:, 2 * fp4 : 2 * fp4 + 2, c * 512 : (c + 1) * 512],
                                start=False,
                                stop=False,
                                perf_mode=DR,
                            )

                # - scores: one fused (k.q + bias) DR matmul per (kt, 512q);
                #   per-kt exp on ping-ponged [128,N] tiles; v interleaved -
                v8 = sb_v.tile([128, KT, 128], FP8, tag="v8", name="v8")
                sig_r = sigs.pop(h)
                for kt in range(2, KT):
                    sc = ps_sc.tile([128, N], F32, tag="ps_sc", name=f"sc{kt}")
                    for c in range(2):
                        nc.tensor.matmul(
                            sc[:, c * 512 : (c + 1) * 512],
                            ki_lhs(par, kt),
                            qb_rhs(parq, kt, c),
                            start=True,
                            stop=True,
                            perf_mode=DR,
                        )
                    if kt == 2:
                        # ACT<->DVE balance: c1's exp via the Schraudolph
                        # int-exp on DVE (int32 convert + bitcast to f32)
                        nc.scalar.activation(
                            out=PT[:, kt, 0:512],
                            in_=sc[:, 0:512],
                            func=AF.Exp,
                            scale=ES,
                        )
                        ti = sb_dr.tile([128, 512], mybir.dt.int32,
                                        tag="ti", name="ti")
                        nc.vector.tensor_scalar(
                            ti, sc[:, 512:N], XA * ES, XB,
                            mybir.AluOpType.mult, mybir.AluOpType.add,
                        )
                        nc.vector.tensor_copy(
                            out=PT[:, kt, 512:N],
                            in_=ti[:, :].bitcast(F32),
                        )
                    else:
                        nc.scalar.activation(
                            out=PT[:, kt, :],
                            in_=sc,
                            func=AF.Exp,
                            scale=ES,
                        )
                    if kt == 3 or kt == 5:
                        # v projection chunk (natural [k-token, dh] layout)
                        c = (kt - 3) // 2
                        vr = ps_tl.tile(
                            [128, 4, 128], F32, tag="ps_tl", name=f"vr{c}"
                        )
                        for nb4 in range(4):
                            nb = 4 * c + nb4
                            for ktp in range(KTP):
                                nc.tensor.matmul(
                                    vr[:, nb4, :],
                                    XT8[:, 2 * ktp : 2 * ktp + 2, nb * 128 : (nb + 1) * 128],
                                    wt[:, 2, ktp, :, :],
                                    start=(ktp == 0),
                                    stop=(ktp == KTP - 1),
                                    perf_mode=DR,
                                )
                        nc.vector.tensor_copy(
                            out=v8[:, 4 * c : 4 * c + 4, :], in_=vr
                        )
                nc.sync.dma_start(out=XR8[:, h, :, :], in_=xhl_d[h])

                if h + 1 < H:
                    head_prologue(h + 1)

                # - denominators -> rbb = 1/denom; av = v8^T @ PT;
                #   t1 = av*sig_r (DVE); ATT = t1*rbb (Pool, sbuf) -
                # last head: narrow first 128 q columns so nt0's ff
                # dependency binds early (shortens the FF fill chain)
                rbb = sb_dr.tile([128, N], F32, tag="rbb", name="rbb")
                spans = [(0, 512), (512, 1024)]
                if h == H - 1:
                    spans = [(0, 128), (128, 512), (512, 1024)]
                for lo, hi in spans:
                    dn = ps_tl.tile([128, hi - lo], F32, tag="ps_tl", name="dn")
                    for ktp in range(KTP):
                        nc.tensor.matmul(
                            dn,
                            ones2,
                            PT[:, 2 * ktp : 2 * ktp + 2, lo:hi],
                            start=(ktp == 0),
                            stop=(ktp == KTP - 1),
                            perf_mode=DR,
                        )
                    nc.vector.reciprocal(rbb[:, lo:hi], dn)
                    av = ps_tl.tile([128, hi - lo], F32, tag="ps_tl", name="av")
                    for ktp in range(KTP):
                        nc.tensor.matmul(
                            av,
                            v8[:, 2 * ktp : 2 * ktp + 2, :],
                            PT[:, 2 * ktp : 2 * ktp + 2, lo:hi],
                            start=(ktp == 0),
                            stop=(ktp == KTP - 1),
                            perf_mode=DR,
                        )
                    t1 = sb_dr.tile([128, hi - lo], BF16, tag="t1", name="t1")
                    nc.vector.tensor_mul(t1, av, sig_r[:, lo:hi])
                    nc.gpsimd.tensor_mul(
                        ATT[:, h, lo:hi], t1, rbb[:, lo:hi]
                    )

            # ---- output projection + residual + LayerNorm ----
            for nt in range(N // 128):
                ffs = []
                stats = sb_st.tile([128, 2, 6], F32, tag="stats", name="stats")
                r4 = nt % 4
                ffsc = None
                if r4 >= 2:
                    ffsc = ps_sc.tile([128, N], F32, tag="ps_sc", name="ffp")
                for c in range(2):
                    if nt == 0:
                        ff = ff_pre[c]
                    elif r4 == 0:
                        ff = ps_pr.tile([128, 512], F32, tag="ps_pr", name=f"ff{c}")
                    elif r4 == 1:
                        ff = ps_tl.tile([128, 512], F32, tag="ps_tl", name=f"ff{c}")
                    else:
                        ff = ffsc[:, c * 512 : (c + 1) * 512]
                    ffs.append(ff)
                    if nt > 0:
                        # residual: ff = I^T@xhi + I^T@xlo (fp8 hi/lo DR)
                        nc.tensor.matmul(
                            ff,
                            id_res(),
                            XR8[:, nt, :, c * 512 : (c + 1) * 512],
                            start=True,
                            stop=False,
                            perf_mode=DR,
                        )
                    fp0 = KTP - 1 if nt == 0 else 0
                    for fp4 in range(fp0, KTP):
                        nc.tensor.matmul(
                            ff,
                            ATT[:, 2 * fp4 : 2 * fp4 + 2, nt * 128 : (nt + 1) * 128],
                            WFF8[:, 2 * fp4 : 2 * fp4 + 2, c * 512 : (c + 1) * 512],
                            start=False,
                            stop=(fp4 == KTP - 1),
                            perf_mode=DR,
                        )
                    if use_bff:
                        nc.vector.tensor_add(
                            ff, ff, bffb[:, c * 512 : (c + 1) * 512]
                        )
                    nc.vector.bn_stats(out=stats[:, c, :], in_=ff)
                mv = sb_st.tile([128, 2], F32, tag="mv", name="mv")
                nc.vector.bn_aggr(out=mv, in_=stats)
                rstd = sb_st.tile([128, 1], F32, tag="rstd", name="rstd")
                nc.scalar.activation(
                    out=rstd,
                    in_=mv[:, 1:2],
                    func=AF.Abs_reciprocal_sqrt,
                    bias=eps_t,
                    scale=1.0,
                )
                mb = sb_st.tile([128, 1], F32, tag="mb", name="mb")
                nc.gpsimd.tensor_scalar(
                    mb, mv[:, 0:1], -1.0, rstd,
                    mybir.AluOpType.mult, mybir.AluOpType.mult,
                )
                o = sb_o.tile([128, D], BF16, tag="o", name="o")
                norm_spans = (
                    [(0, D, ffsc)] if ffsc is not None and nt < 7
                    else [(0, 512, ffs[0]), (512, D, ffs[1])]
                )
                for lo, hi, ff in norm_spans:
                    nc.scalar.activation(
                        out=o[:, lo:hi],
                        in_=ff,
                        func=AF.Identity,
                        bias=mb,
                        scale=rstd,
                    )
                    if use_lng:
                        nc.vector.tensor_mul(
                            o[:, lo:hi], o[:, lo:hi], lngb[:, lo:hi]
                        )
                    if use_lnb:
                        nc.vector.tensor_add(
                            o[:, lo:hi], o[:, lo:hi], lnbb[:, lo:hi]
                        )
                    nc.sync.dma_start(
                        out=out_d[nt * 128 : (nt + 1) * 128, lo:hi],
                        in_=o[:, lo:hi],
                    )

    nc.finalize()
    return nc


def get_nc(flags=(False, False, False, False)):
    if flags not in _cache:
        _cache[flags] = _build(flags)
    return _cache[flags]


def _fp8(a):
    return np.asarray(a, dtype=np.float32).astype(FP8NP)


def kernel(x, mask, bias, gamma_f, W_att, W_ff, b_ff, ln_g, ln_b):
    x = np.asarray(x, dtype=np.float32)
    mask = np.asarray(mask)
    bias = np.asarray(bias, dtype=np.float32)
    gamma_f = np.asarray(gamma_f, dtype=np.float32)
    W_att = np.asarray(W_att, dtype=np.float32)
    W_ff = np.asarray(W_ff, dtype=np.float32)
    b_ff = np.asarray(b_ff, dtype=np.float32)
    ln_g = np.asarray(ln_g, dtype=np.float32)
    ln_b = np.asarray(ln_b, dtype=np.float32)

    general_gamma = not np.all(gamma_f == 1.0)
    use_bff = bool(np.any(b_ff != 0.0))
    use_lng = not np.all(ln_g == 1.0)
    use_lnb = bool(np.any(ln_b != 0.0))
    flags = (general_gamma, use_bff, use_lng, use_lnb)
    nc = get_nc(flags)

    # watt8[h, p, j, ktp, i, fcol] = W_att[ktp*256 + i*128 + p, sect_j + h*128
    #   + fcol] * scale_j   (j: 0=q 1=k 2=v 3=g)
    w4 = W_att.reshape(KTP, 2, 128, 4, H, DH)  # [ktp, i, p, sect, h, fcol]
    watt8 = np.empty((H, 128, 4, KTP, 2, 128), dtype=FP8NP)
    scales = (SQ, SQ, SV, SV)
    for j in range(4):
        # -> [h, p, ktp, i, fcol]
        sect = np.transpose(w4[:, :, :, j, :, :], (3, 2, 0, 1, 4))
        watt8[:, :, j, :, :, :] = _fp8(sect * scales[j])

    wff8 = _fp8(4.0 * W_ff.reshape(H, 128, D).transpose(1, 0, 2))

    valid = ~mask[:, 0, :, :]  # [B, N, N] True where kept

    in_maps = []
    for b in range(B):
        # x^T tiled: XT8[p, kt, n] = x[n, kt*128 + p]
        xt8 = _fp8(np.ascontiguousarray(
            x[b].T.reshape(KT, 128, N).transpose(1, 0, 2)
        ))
        # BT[p, kt, q] = (gamma*bias[q, kt*128+p] + OFF)*SU, masked -> -240
        btr = np.ascontiguousarray(bias[b].T)  # [k, q]
        btr = btr.reshape(KT, 128, N).transpose(1, 0, 2)  # [p, kt, q]
        vtr = np.ascontiguousarray(valid[b].T).reshape(KT, 128, N).transpose(1, 0, 2)
        if general_gamma:
            bt8 = np.empty((H, 128, KT, N), dtype=FP8NP)
            for h in range(H):
                uh = np.clip((gamma_f[h] * btr + OFF) * SU, -239.0, 239.0)
                bt8[h] = np.where(vtr, uh, np.float32(-240.0)).astype(FP8NP)
        else:
            uh = np.clip((btr + OFF) * SU, -239.0, 239.0)
            bt8 = np.where(vtr, uh, np.float32(-240.0)).astype(FP8NP)
        # residual hi/lo: x16 = hi + lo in fp8
        x16 = SA * x[b]
        hi = x16.astype(FP8NP)
        lo = (x16 - hi.astype(np.float32)).astype(FP8NP)
        xhl = np.stack(
            [hi.reshape(KT, 128, D), lo.reshape(KT, 128, D)], axis=2
        )  # [KT, 128, 2, D]
        im = {
            "xt8": xt8,
            "bt": bt8,
            "watt": watt8,
            "wff": wff8,
            "xhl": np.ascontiguousarray(xhl),
        }
        if use_bff:
            im["bff"] = SA * b_ff.reshape(1, D)
        if use_lng:
            im["lng"] = ln_g.reshape(1, D)
        if use_lnb:
            im["lnb"] = ln_b.reshape(1, D)
        in_maps.append(im)

    res = run_bass_kernel_spmd(nc, in_maps, core_ids=list(range(B)))
    out = np.stack([res.results[b]["out"] for b in range(B)], axis=0)
    return out.astype(np.float32)
